# revision 23
# baseline (speedup 1.0000x reference)
"""CodebookLinear TRN2 kernel, v5.

Reference computation (jax):
    W = codebook[indices].reshape(-1)[:4096*4096].reshape(4096, 4096)   # [out, in]
    out = einsum('bsi,oi->bso', x, W) + bias

Distribution: 8 NeuronCores, column-parallel over out_features (each core
owns 512 output features and all 8192 tokens), no collectives.

The W reconstruction is the hard part.  HW-probed rates for the three
gather primitives (per 8-value codebook block, per core):
  - gpsimd.ap_gather:        3.5 ns/block of Pool time (28.1 ns per
    group-index, 8 Q7 cores in parallel; invariant in d and num_idxs)
  - gpsimd.indirect_dma_start: one dynamic offset per partition per call
    is all the HW honors -> 1.1 us SWDGE launch per 128 blocks (8.6 ns/b)
  - gpsimd.dma_gather:       ~10 ns/block drain-bound, and lives in a
    different ucode library than ap_gather (6 us IRAM swaps)
so ap_gather wins and the kernel is Pool-bound at ~920 us; everything
else is structured to hide completely under it:
  - x^T f32 loads via HWDGE on the Act queue (Pool stays gather-only),
    cast to bf16 on the DVE/Act engines, double-buffered.
  - Matmuls run in two K-halves so PSUM accumulates 16 k-tiles per token
    tile; half-0 is held as a bf16 partial in SBUF (8 MB, no DRAM
    roundtrip) and added during the half-1 drain.  Accumulation groups
    are back-to-back (223 ns/matmul floor measured).
  - Bias is preloaded into PSUM via a K=1 ones x bias matmul.
  - W^T k-tiles become available every ~29 us and the PE paces behind
    them; the tail after the last gather is one matmul group + drain.

Host side only shards/reshapes: x is transposed and row-permuted within
each 128-row k-tile to match ap_gather's channel order (pure
permutation), indices are converted to int16 and pre-permuted into the
wrapped per-group interleaved layout the gather consumes, the codebook
is re-laid-out as the per-partition column table ap_gather reads
(data[p, k] = cb[k, (p>>1)&7] -- replaces ~145 us of on-device
transpose/replication setup that serialized before the first gather),
bias sliced.

Measured on 8 axon TRN2 cores: HW exec ~1.20 ms, rel err 2.63e-03.

Index/partition math (per core, o local in [0, 512)):
  Within k-tile it, SBUF partition p holds contraction row
      i = 128*it + sigma(p),  sigma(p) = 8*(2*(p>>4) + (p&1)) + ((p>>1)&7)
  so  j(i) = 16*it + 2*g + h,  k(i) = (p>>1)&7,  g = p>>4,  h = p&1.
  group g's list for k-tile it:  L[n = 2*o + h] = idx[o, 16*it + 2*g + h]
  wrapped storage:               idxw[16*g + q, it, f] = L[16*f + q]
  gather:  g2[p, n] = data[p, L[g(p)][n]] = cb[idx[o(n), j], k(p)]
  select:  W^T[p, o] = g2[p, 2*o + (p&1)]
"""

import sys

for _p in ("/opt/trn_rl_repo",):
    if _p not in sys.path:
        sys.path.insert(0, _p)

import numpy as np

import concourse.bacc as bacc
import concourse.mybir as mybir
import concourse.tile as tile
from concourse.bass_utils import run_bass_kernel_spmd
from concourse.masks import make_identity

# Problem constants
OUT_F = 4096
IN_F = 4096
KCB = 4096          # codebook entries
BS = 8              # block size
JB = IN_F // BS     # 512 blocks per W row
B, S = 4, 2048
T = B * S           # 8192 tokens

S_O = 8
O_LOC = OUT_F // S_O   # 512
T_LOC = T              # 8192

P = 128
NIT = IN_F // P        # 32 k-tiles
NTT = T_LOC // P       # 64 token tiles
TT4 = 4                # token tiles per x load
NG = NTT // TT4        # 16 token-tile groups
FW = 2 * O_LOC // 16   # wrapped index columns per k-tile
# Decreasing K-chunks: chunk c's matmuls (16 groups x ~1us/k-tile) hide
# under chunk c+1's gathers (28.3us/k-tile); only the last 2-tile chunk's
# matmuls run after the final gather.
CH = [12, 8, 6, 4, 2]
CSTART = [sum(CH[:i]) for i in range(len(CH))]
NCH = len(CH)

# partition -> within-tile contraction row
_p_ar = np.arange(P)
SIGMA = (8 * (2 * (_p_ar >> 4) + (_p_ar & 1)) + ((_p_ar >> 1) & 7)).astype(np.int64)

_nc_cache = None
last_result = None     # BassKernelResults of the most recent run (for test.py)


def _ensure_ntff_hook():
    """The image's antenv lacks axon_hooks; provide it so tracing works."""
    try:
        import antenv.axon_hooks  # noqa: F401
        return
    except ImportError:
        pass
    import types

    import antenv

    mod = types.ModuleType("antenv.axon_hooks")
    _hook = [None]

    def set_axon_ntff_profile_hook(h):
        _hook[0] = h

    def get_axon_ntff_profile_hook():
        if _hook[0] is None:
            try:
                from trn_agent_boot.trn_boot import _ntff_profile_via_ctypes

                _hook[0] = _ntff_profile_via_ctypes("/opt/axon/libaxon_pjrt.so")
            except Exception:
                return None
        return _hook[0]

    mod.get_axon_ntff_profile_hook = get_axon_ntff_profile_hook
    mod.set_axon_ntff_profile_hook = set_axon_ntff_profile_hook
    sys.modules["antenv.axon_hooks"] = mod
    antenv.axon_hooks = mod


def build_nc():
    nc = bacc.Bacc("TRN2", target_bir_lowering=False, debug=False)
    xT = nc.dram_tensor("xT", [NIT, P, T_LOC], mybir.dt.float32, kind="ExternalInput")
    idxw = nc.dram_tensor("idxw", [P, NIT * FW], mybir.dt.int16, kind="ExternalInput")
    # Codebook pre-laid-out on host: data_d[p, k] = cb[k, (p>>1)&7]
    data_d = nc.dram_tensor("data", [P, KCB], mybir.dt.float32, kind="ExternalInput")
    bias = nc.dram_tensor("bias", [1, O_LOC], mybir.dt.float32, kind="ExternalInput")
    mask = nc.dram_tensor("mask", [P, 1], mybir.dt.uint8, kind="ExternalInput")
    out = nc.dram_tensor("out", [T_LOC, O_LOC], mybir.dt.float32, kind="ExternalOutput")
    part_dram = nc.dram_tensor("part_scratch", [T_LOC, O_LOC], mybir.dt.bfloat16)

    with tile.TileContext(nc) as tc:
        with (
            tc.tile_pool(name="const", bufs=1) as constp,
            tc.tile_pool(name="wt", bufs=1) as wtp,
            tc.tile_pool(name="g2p", bufs=2) as g2p,
            tc.tile_pool(name="xfp", bufs=2) as xfp,
            tc.tile_pool(name="xbp", bufs=3) as xbp,
            tc.tile_pool(name="pbp", bufs=4) as pbp,
            tc.tile_pool(name="outp", bufs=4) as outp,
            tc.tile_pool(name="psmm", bufs=4, space="PSUM") as psmm,
        ):
            ones_row = constp.tile([1, P], mybir.dt.float32)
            nc.gpsimd.memset(ones_row[:], 1.0)
            bias_row = constp.tile([1, O_LOC], mybir.dt.float32)
            nc.sync.dma_start(out=bias_row[:], in_=bias[:, :])
            mask_t = constp.tile([P, 1], mybir.dt.uint8)
            nc.sync.dma_start(out=mask_t[:], in_=mask[:, :])
            idf = constp.tile([P, P], mybir.dt.float32)
            make_identity(nc, idf[:])
            idb = constp.tile([P, P], mybir.dt.bfloat16)
            nc.scalar.copy(out=idb[:], in_=idf[:])

            data = constp.tile([P, KCB], mybir.dt.float32)
            nc.sync.dma_start(out=data[:], in_=data_d[:, :])
            idxt = constp.tile([P, NIT * FW], mybir.dt.int16)
            nc.sync.dma_start(out=idxt[:], in_=idxw[:, :])

            # Resident W^T bf16 [p = sigma-row, it, o]
            WT = wtp.tile([P, NIT, O_LOC], mybir.dt.bfloat16)
            mask_bc = mask_t[:, 0:1].to_broadcast([P, O_LOC])

            def gather_tile(it):
                """ap_gather k-tile `it`; select/cast into WT[:, it, :]."""
                g2 = g2p.tile([P, 2 * O_LOC], mybir.dt.float32)
                nc.gpsimd.ap_gather(
                    out_ap=g2[:, :],
                    in_ap=data[:, :],
                    idxs_ap=idxt[:, it * FW : (it + 1) * FW],
                    channels=P,
                    num_elems=KCB,
                    d=1,
                    num_idxs=2 * O_LOC,
                )
                g2_s = g2[:, :].rearrange("p (o s) -> p o s", s=2)
                nc.vector.tensor_copy(out=WT[:, it, :], in_=g2_s[:, :, 0])
                nc.vector.copy_predicated(
                    out=WT[:, it, :], mask=mask_bc, data=g2_s[:, :, 1]
                )

            def xload(c, g):
                """HWDGE f32 load of x^T for chunk c's k-tiles, token tiles
                [4g, 4g+4), then bf16 cast on Act.  Returns the bf16 tile."""
                s = CH[c]
                xf = xfp.tile([P, CH[0], TT4 * P], mybir.dt.float32, name="xf")
                nc.scalar.dma_start(
                    out=xf[:, :s, :],
                    in_=xT[CSTART[c] : CSTART[c] + s, :,
                          g * TT4 * P : (g + 1) * TT4 * P].rearrange(
                        "a p t -> p a t"
                    ),
                )
                xb = xbp.tile([P, CH[0], TT4 * P], mybir.dt.bfloat16, name="xb")
                nc.scalar.copy(out=xb[:, :s, :], in_=xf[:, :s, :])
                return xb

            def mm_group(c, g, xb):
                """Matmuls for token tiles [4g, 4g+4) of K-chunk c.  PSUM is
                seeded with bias (chunk 0) or the running bf16 partial
                (exact identity-matmul preload); the partial roundtrips
                through DRAM between chunks."""
                pbr = []
                for u in range(TT4):
                    tt = g * TT4 + u
                    if c > 0:
                        pb = pbp.tile([P, O_LOC], mybir.dt.bfloat16, name="pbr")
                        nc.sync.dma_start(
                            out=pb[:], in_=part_dram[tt * P : (tt + 1) * P, :]
                        )
                        pbr.append(pb)
                for u in range(TT4):
                    tt = g * TT4 + u
                    ps = psmm.tile([P, O_LOC], mybir.dt.float32)
                    if c == 0:
                        nc.tensor.matmul(
                            out=ps[:],
                            lhsT=ones_row[:, :],
                            rhs=bias_row[:, :],
                            start=True,
                            stop=False,
                        )
                    else:
                        nc.tensor.matmul(
                            out=ps[:],
                            lhsT=idb[:, :],
                            rhs=pbr[u][:],
                            start=True,
                            stop=False,
                        )
                    for itl in range(CH[c]):
                        nc.tensor.matmul(
                            out=ps[:],
                            lhsT=xb[:, itl, u * P : (u + 1) * P],
                            rhs=WT[:, CSTART[c] + itl, :],
                            start=False,
                            stop=(itl == CH[c] - 1),
                        )
                    if c < NCH - 1:
                        pb = pbp.tile([P, O_LOC], mybir.dt.bfloat16, name="pbw")
                        nc.scalar.copy(out=pb[:], in_=ps[:])
                        nc.sync.dma_start(
                            out=part_dram[tt * P : (tt + 1) * P, :], in_=pb[:]
                        )
                    else:
                        outt = outp.tile([P, O_LOC], mybir.dt.float32)
                        nc.vector.tensor_copy(out=outt[:, :], in_=ps[:])
                        nc.sync.dma_start(
                            out=out[tt * P : (tt + 1) * P, :], in_=outt[:]
                        )

            # ---- program ----
            # All gathers up front: Pool runs only these; every other queue
            # is paced by tile-pool buffer reuse and dataflow deps.
            for it in range(NIT):
                gather_tile(it)
            for c in range(NCH):
                for g in range(NG):
                    xb = xload(c, g)
                    mm_group(c, g, xb)

    nc.compile()
    return nc


def _get_nc():
    global _nc_cache
    if _nc_cache is None:
        _nc_cache = build_nc()
    return _nc_cache


def _wrap_indices(idx_local):
    """[O_LOC, JB] int -> wrapped interleaved int16 [P, NIT*FW]."""
    arr = idx_local.reshape(O_LOC, NIT, 8, 2)        # [o, it, g, h]
    L = arr.transpose(2, 1, 0, 3).reshape(8, NIT, 2 * O_LOC)   # [g, it, n=2o+h]
    Lw = L.reshape(8, NIT, 2 * O_LOC // 16, 16)      # [g, it, f, q]
    w = Lw.transpose(0, 3, 1, 2).reshape(P, NIT * FW)
    return np.ascontiguousarray(w.astype(np.int16))


def make_in_maps(x, codebook, indices, bias):
    x = np.asarray(x, dtype=np.float32).reshape(T, IN_F)
    xT_full = np.ascontiguousarray(x.T)  # [IN_F, T]
    # permute contraction rows within each 128-tile to the sigma order,
    # then tile as [it, p, t]
    xT_t = np.ascontiguousarray(xT_full.reshape(NIT, P, T)[:, SIGMA, :])
    idx2d = np.asarray(indices).astype(np.int64).reshape(OUT_F, JB)
    cb = np.asarray(codebook, dtype=np.float32)
    # data[p, k] = cb[k, (p>>1)&7] -- pure re-layout of the codebook
    data_np = np.ascontiguousarray(cb[:, (np.arange(P) >> 1) & 7].T)
    b = np.asarray(bias, dtype=np.float32)
    mask_np = (np.arange(P) % 2).astype(np.uint8).reshape(P, 1)

    in_maps = []
    for c in range(8):
        in_maps.append(
            {
                "xT": xT_t,
                "idxw": _wrap_indices(idx2d[c * O_LOC : (c + 1) * O_LOC]),
                "data": data_np,
                "bias": np.ascontiguousarray(
                    b[c * O_LOC : (c + 1) * O_LOC]
                ).reshape(1, O_LOC),
                "mask": mask_np,
            }
        )
    return in_maps


def assemble(outs):
    full = np.empty((T, OUT_F), dtype=np.float32)
    for c in range(8):
        full[:, c * O_LOC : (c + 1) * O_LOC] = outs[c]["out"]
    return full.reshape(B, S, OUT_F)


def kernel(x, codebook, indices, bias):
    global last_result
    _ensure_ntff_hook()
    nc = _get_nc()
    in_maps = make_in_maps(x, codebook, indices, bias)
    last_result = run_bass_kernel_spmd(nc, in_maps, core_ids=list(range(8)))
    return assemble(last_result.results)


# revision 27
# speedup vs baseline: 1.1177x; 1.1177x over previous
"""CodebookLinear TRN2 kernel, v5.

Reference computation (jax):
    W = codebook[indices].reshape(-1)[:4096*4096].reshape(4096, 4096)   # [out, in]
    out = einsum('bsi,oi->bso', x, W) + bias

Distribution: 8 NeuronCores, column-parallel over out_features (each core
owns 512 output features and all 8192 tokens), no collectives.

The W reconstruction is the hard part.  HW-probed rates for the three
gather primitives (per 8-value codebook block, per core):
  - gpsimd.ap_gather:        3.5 ns/block of Pool time (28.1 ns per
    group-index, 8 Q7 cores in parallel; invariant in d and num_idxs)
  - gpsimd.indirect_dma_start: one dynamic offset per partition per call
    is all the HW honors -> 1.1 us SWDGE launch per 128 blocks (8.6 ns/b)
  - gpsimd.dma_gather:       ~10 ns/block drain-bound, and lives in a
    different ucode library than ap_gather (6 us IRAM swaps)
so ap_gather wins and the kernel is Pool-bound at ~920 us; everything
else is structured to hide completely under it:
  - x^T f32 loads via HWDGE on the Act queue (Pool stays gather-only),
    cast to bf16 on the DVE/Act engines, double-buffered.
  - Matmuls run in two K-halves so PSUM accumulates 16 k-tiles per token
    tile; half-0 is held as a bf16 partial in SBUF (8 MB, no DRAM
    roundtrip) and added during the half-1 drain.  Accumulation groups
    are back-to-back (223 ns/matmul floor measured).
  - Bias is preloaded into PSUM via a K=1 ones x bias matmul.
  - W^T k-tiles become available every ~29 us and the PE paces behind
    them; the tail after the last gather is one matmul group + drain.

Host side only shards/reshapes: x is transposed and row-permuted within
each 128-row k-tile to match ap_gather's channel order (pure
permutation), indices are converted to int16 and pre-permuted into the
wrapped per-group interleaved layout the gather consumes, the codebook
is re-laid-out as the per-partition column table ap_gather reads
(data[p, k] = cb[k, (p>>1)&7] -- replaces ~145 us of on-device
transpose/replication setup that serialized before the first gather),
bias sliced.

Measured on 8 axon TRN2 cores: HW exec ~1.20 ms, rel err 2.63e-03.

Index/partition math (per core, o local in [0, 512)):
  Within k-tile it, SBUF partition p holds contraction row
      i = 128*it + sigma(p),  sigma(p) = 8*(2*(p>>4) + (p&1)) + ((p>>1)&7)
  so  j(i) = 16*it + 2*g + h,  k(i) = (p>>1)&7,  g = p>>4,  h = p&1.
  group g's list for k-tile it:  L[n = 2*o + h] = idx[o, 16*it + 2*g + h]
  wrapped storage:               idxw[16*g + q, it, f] = L[16*f + q]
  gather:  g2[p, n] = data[p, L[g(p)][n]] = cb[idx[o(n), j], k(p)]
  select:  W^T[p, o] = g2[p, 2*o + (p&1)]
"""

import sys

for _p in ("/opt/trn_rl_repo",):
    if _p not in sys.path:
        sys.path.insert(0, _p)

import numpy as np

import concourse.bacc as bacc
import concourse.mybir as mybir
import concourse.tile as tile
from concourse.bass_utils import run_bass_kernel_spmd

# Problem constants
OUT_F = 4096
IN_F = 4096
KCB = 4096          # codebook entries
BS = 8              # block size
JB = IN_F // BS     # 512 blocks per W row
B, S = 4, 2048
T = B * S           # 8192 tokens

S_O = 8
O_LOC = OUT_F // S_O   # 512
T_LOC = T              # 8192

P = 128
NIT = IN_F // P        # 32 k-tiles
NTT = T_LOC // P       # 64 token tiles
NH = 2                 # K-halves
ITH = NIT // NH        # 16 k-tiles per half
TT4 = 4                # token tiles per x load
NG = NTT // TT4        # 16 x-load groups per half
FW = 2 * O_LOC // 16   # wrapped index columns per k-tile

# partition -> within-tile contraction row
_p_ar = np.arange(P)
SIGMA = (8 * (2 * (_p_ar >> 4) + (_p_ar & 1)) + ((_p_ar >> 1) & 7)).astype(np.int64)

_nc_cache = None
last_result = None     # BassKernelResults of the most recent run (for test.py)


def _ensure_ntff_hook():
    """The image's antenv lacks axon_hooks; provide it so tracing works."""
    try:
        import antenv.axon_hooks  # noqa: F401
        return
    except ImportError:
        pass
    import types

    import antenv

    mod = types.ModuleType("antenv.axon_hooks")
    _hook = [None]

    def set_axon_ntff_profile_hook(h):
        _hook[0] = h

    def get_axon_ntff_profile_hook():
        if _hook[0] is None:
            try:
                from trn_agent_boot.trn_boot import _ntff_profile_via_ctypes

                _hook[0] = _ntff_profile_via_ctypes("/opt/axon/libaxon_pjrt.so")
            except Exception:
                return None
        return _hook[0]

    mod.get_axon_ntff_profile_hook = get_axon_ntff_profile_hook
    mod.set_axon_ntff_profile_hook = set_axon_ntff_profile_hook
    sys.modules["antenv.axon_hooks"] = mod
    antenv.axon_hooks = mod


def build_nc():
    nc = bacc.Bacc("TRN2", target_bir_lowering=False, debug=False)
    xT = nc.dram_tensor("xT", [NIT, P, T_LOC], mybir.dt.float32, kind="ExternalInput")
    idxw = nc.dram_tensor("idxw", [P, NIT * FW], mybir.dt.int16, kind="ExternalInput")
    # Codebook pre-laid-out on host: data_d[p, k] = cb[k, (p>>1)&7]
    data_d = nc.dram_tensor("data", [P, KCB], mybir.dt.float32, kind="ExternalInput")
    bias = nc.dram_tensor("bias", [1, O_LOC], mybir.dt.float32, kind="ExternalInput")
    mask = nc.dram_tensor("mask", [P, 1], mybir.dt.uint8, kind="ExternalInput")
    out = nc.dram_tensor("out", [T_LOC, O_LOC], mybir.dt.float32, kind="ExternalOutput")
    part_dram = nc.dram_tensor("part_scratch", [T_LOC, O_LOC], mybir.dt.bfloat16)

    with tile.TileContext(nc) as tc:
        with (
            tc.tile_pool(name="const", bufs=1) as constp,
            tc.tile_pool(name="wt", bufs=1) as wtp,
            tc.tile_pool(name="g2p", bufs=2) as g2p,
            tc.tile_pool(name="xfp", bufs=2) as xfp,
            tc.tile_pool(name="xbp", bufs=3) as xbp,
            tc.tile_pool(name="pbp", bufs=4) as pbp,
            tc.tile_pool(name="outp", bufs=4) as outp,
            tc.tile_pool(name="psmm", bufs=4, space="PSUM") as psmm,
        ):
            ones_row = constp.tile([1, P], mybir.dt.float32)
            nc.gpsimd.memset(ones_row[:], 1.0)
            bias_row = constp.tile([1, O_LOC], mybir.dt.float32)
            nc.sync.dma_start(out=bias_row[:], in_=bias[:, :])
            mask_t = constp.tile([P, 1], mybir.dt.uint8)
            nc.sync.dma_start(out=mask_t[:], in_=mask[:, :])

            data = constp.tile([P, KCB], mybir.dt.float32)
            nc.sync.dma_start(out=data[:], in_=data_d[:, :])
            idxt = constp.tile([P, NIT * FW], mybir.dt.int16)
            nc.sync.dma_start(out=idxt[:], in_=idxw[:, :])

            # Resident W^T bf16 [p = sigma-row, it, o]
            WT = wtp.tile([P, NIT, O_LOC], mybir.dt.bfloat16)
            mask_bc = mask_t[:, 0:1].to_broadcast([P, O_LOC])

            def gather_tile(it):
                """ap_gather k-tile `it`; select/cast into WT[:, it, :]."""
                g2 = g2p.tile([P, 2 * O_LOC], mybir.dt.float32)
                nc.gpsimd.ap_gather(
                    out_ap=g2[:, :],
                    in_ap=data[:, :],
                    idxs_ap=idxt[:, it * FW : (it + 1) * FW],
                    channels=P,
                    num_elems=KCB,
                    d=1,
                    num_idxs=2 * O_LOC,
                )
                g2_s = g2[:, :].rearrange("p (o s) -> p o s", s=2)
                nc.vector.tensor_copy(out=WT[:, it, :], in_=g2_s[:, :, 0])
                nc.vector.copy_predicated(
                    out=WT[:, it, :], mask=mask_bc, data=g2_s[:, :, 1]
                )

            def xload(h, g):
                """HWDGE f32 load of x^T k-half h, token tiles [4g, 4g+4),
                then bf16 cast on Act.  Returns the bf16 tile."""
                xf = xfp.tile([P, ITH, TT4 * P], mybir.dt.float32, name="xf")
                nc.scalar.dma_start(
                    out=xf[:, :, :],
                    in_=xT[h * ITH : (h + 1) * ITH, :,
                          g * TT4 * P : (g + 1) * TT4 * P].rearrange(
                        "a p t -> p a t"
                    ),
                )
                xb = xbp.tile([P, ITH, TT4 * P], mybir.dt.bfloat16, name="xb")
                nc.scalar.copy(out=xb[:, :, :], in_=xf[:, :, :])
                return xb

            def mm_group(h, g, xb):
                """Matmuls for token tiles [4g, 4g+4) of K-half h."""
                for u in range(TT4):
                    tt = g * TT4 + u
                    ps = psmm.tile([P, O_LOC], mybir.dt.float32)
                    if h == 0:
                        nc.tensor.matmul(
                            out=ps[:],
                            lhsT=ones_row[:, :],
                            rhs=bias_row[:, :],
                            start=True,
                            stop=False,
                        )
                    for itl in range(ITH):
                        nc.tensor.matmul(
                            out=ps[:],
                            lhsT=xb[:, itl, u * P : (u + 1) * P],
                            rhs=WT[:, h * ITH + itl, :],
                            start=(h == 1 and itl == 0),
                            stop=(itl == ITH - 1),
                        )
                    if h == 0:
                        pb = pbp.tile([P, O_LOC], mybir.dt.bfloat16, name="pb")
                        nc.scalar.copy(out=pb[:], in_=ps[:])
                        nc.sync.dma_start(
                            out=part_dram[tt * P : (tt + 1) * P, :], in_=pb[:]
                        )
                    else:
                        pb = pbp.tile([P, O_LOC], mybir.dt.bfloat16, name="pb")
                        nc.sync.dma_start(
                            out=pb[:], in_=part_dram[tt * P : (tt + 1) * P, :]
                        )
                        outt = outp.tile([P, O_LOC], mybir.dt.float32)
                        nc.vector.tensor_tensor(
                            out=outt[:, :],
                            in0=ps[:],
                            in1=pb[:],
                            op=mybir.AluOpType.add,
                        )
                        nc.sync.dma_start(
                            out=out[tt * P : (tt + 1) * P, :], in_=outt[:]
                        )

            # ---- program ----
            xqueue = []
            for it in range(ITH):
                gather_tile(it)
                if it % 4 == 3:
                    xqueue.append(xload(0, it // 4))
            for g in range(NG):
                gather_tile(ITH + g)
                if g + 4 < NG:
                    xqueue.append(xload(0, g + 4))
                elif g + 4 < 2 * NG:
                    xqueue.append(xload(1, g + 4 - NG))
                mm_group(0, g, xqueue[g])
            for g in range(NG):
                if g + 4 < NG:
                    xqueue.append(xload(1, g + 4))
                mm_group(1, g, xqueue[NG + g])

    nc.compile()
    return nc


def _get_nc():
    global _nc_cache
    if _nc_cache is None:
        _nc_cache = build_nc()
    return _nc_cache


def _wrap_indices(idx_local):
    """[O_LOC, JB] int -> wrapped interleaved int16 [P, NIT*FW]."""
    arr = idx_local.reshape(O_LOC, NIT, 8, 2)        # [o, it, g, h]
    L = arr.transpose(2, 1, 0, 3).reshape(8, NIT, 2 * O_LOC)   # [g, it, n=2o+h]
    Lw = L.reshape(8, NIT, 2 * O_LOC // 16, 16)      # [g, it, f, q]
    w = Lw.transpose(0, 3, 1, 2).reshape(P, NIT * FW)
    return np.ascontiguousarray(w.astype(np.int16))


def make_in_maps(x, codebook, indices, bias):
    x = np.asarray(x, dtype=np.float32).reshape(T, IN_F)
    xT_full = np.ascontiguousarray(x.T)  # [IN_F, T]
    # permute contraction rows within each 128-tile to the sigma order,
    # then tile as [it, p, t]
    xT_t = np.ascontiguousarray(xT_full.reshape(NIT, P, T)[:, SIGMA, :])
    idx2d = np.asarray(indices).astype(np.int64).reshape(OUT_F, JB)
    cb = np.asarray(codebook, dtype=np.float32)
    # data[p, k] = cb[k, (p>>1)&7] -- pure re-layout of the codebook
    data_np = np.ascontiguousarray(cb[:, (np.arange(P) >> 1) & 7].T)
    b = np.asarray(bias, dtype=np.float32)
    mask_np = (np.arange(P) % 2).astype(np.uint8).reshape(P, 1)

    in_maps = []
    for c in range(8):
        in_maps.append(
            {
                "xT": xT_t,
                "idxw": _wrap_indices(idx2d[c * O_LOC : (c + 1) * O_LOC]),
                "data": data_np,
                "bias": np.ascontiguousarray(
                    b[c * O_LOC : (c + 1) * O_LOC]
                ).reshape(1, O_LOC),
                "mask": mask_np,
            }
        )
    return in_maps


def assemble(outs):
    full = np.empty((T, OUT_F), dtype=np.float32)
    for c in range(8):
        full[:, c * O_LOC : (c + 1) * O_LOC] = outs[c]["out"]
    return full.reshape(B, S, OUT_F)


def kernel(x, codebook, indices, bias):
    global last_result
    _ensure_ntff_hook()
    nc = _get_nc()
    in_maps = make_in_maps(x, codebook, indices, bias)
    last_result = run_bass_kernel_spmd(nc, in_maps, core_ids=list(range(8)))
    return assemble(last_result.results)
